# revision 39
# baseline (speedup 1.0000x reference)
"""Trainium2 Bass kernel for nn_C_GAN_NET_9320079032867.

The reference "2-layer LSTM over T steps" has NO cross-timestep recurrence
(writes go to state slot i+1, slot 0 stays zero, slot 1 is overwritten before
it is read), so every (batch, time) token is an independent feed-forward
computation:

    g0 = x @ W_ih0.T               (f-gate of layer 0 provably unused: c=0)
    c0 = sig(i0) * tanh(g0g);  h0 = sig(o0) * tanh(c0)
    out0 = sig(h0 @ W_hh0.T)
    g1 = x @ W_ih1.T + h0 @ W_hh1.T
    c1 = sig(f1) * c0 + sig(i1) * tanh(g1g);  h1 = sig(o1) * tanh(c1)
    out1 = sig(h1 @ W_hh1.T)
    out  = concat(out0, out1)      # [B, T, 4096]

b_ih / b_hh are structurally zero and skipped.

Sharding: data-parallel over batch across 8 cores (2048 tokens per core);
params replicated.

Precision: matmuls run fp8-e4m3 in DoubleRow mode (2 contraction chunks per
instruction, 2x PE throughput) EXCEPT the tanh-gate (g) x-part matmuls
(g0 fully, g1's x half), which stay bf16: tanh passes gate error through at
~1x while sigmoid gates attenuate it ~4x.  Measured max rel err 1.61e-2
vs the 2e-2 gate (host numpy sim of the exact quantization chain predicts
this within ~2%; see acc_sim*.py).

Layout trick: host passes x.T and W.T so gates are computed in transposed
layout gates.T[unit, tok] = W @ x.T with both operands native; h0T/h1T fall
out as the stationary operands of the final z matmuls whose outputs land in
natural [tok, unit] layout for contiguous output DMA. Zero on-chip
transposes.
"""
import os

import numpy as np
import ml_dtypes

import concourse.tile as tile
import concourse.mybir as mybir
from concourse import bacc
from concourse.bass_utils import run_bass_kernel_spmd

# Problem constants (hardcoded per harness contract).
B, T, D, H, L = 128, 128, 512, 512, 2
NCORES = 8
TOK = B * T // NCORES        # tokens per core = 2048
BLK = 512                    # tokens per pipeline block
NB = TOK // BLK              # 4 blocks
G4 = 4 * H                   # 2048 gate units per layer

BF = mybir.dt.bfloat16
F8 = mybir.dt.float8e4
BF_NP = ml_dtypes.bfloat16
F8_NP = ml_dtypes.float8_e4m3   # TRN fp8_exp4 semantics (max +-240)
DR = mybir.MatmulPerfMode.DoubleRow

SIG = mybir.ActivationFunctionType.Sigmoid
TANH = mybir.ActivationFunctionType.Tanh


def _build():
    nc = bacc.Bacc("TRN2", target_bir_lowering=False, debug=False)

    # DRAM I/O (per core).  x transposed [D, TOK]; weights transposed
    # [D|H, units].  fp8 weight packs exclude the bf16 g-gate columns.
    xf8_d = nc.dram_tensor("xf8", [D, TOK], F8, kind="ExternalInput").ap()
    xbf_d = nc.dram_tensor("xbf", [D, TOK], BF, kind="ExternalInput").ap()
    wih0_d = nc.dram_tensor("wih0", [D, 1024], F8, kind="ExternalInput").ap()   # [i,o]
    wih1_d = nc.dram_tensor("wih1", [D, 1536], F8, kind="ExternalInput").ap()   # [i,f,o]
    whh0_d = nc.dram_tensor("whh0", [H, G4], F8, kind="ExternalInput").ap()     # full
    whh1_d = nc.dram_tensor("whh1", [H, G4], F8, kind="ExternalInput").ap()     # full
    wg0_d = nc.dram_tensor("wg0", [D, H], BF, kind="ExternalInput").ap()        # g cols
    wg1x_d = nc.dram_tensor("wg1x", [D, H], BF, kind="ExternalInput").ap()
    out_d = nc.dram_tensor("out", [TOK, 2 * G4], mybir.dt.float32,
                           kind="ExternalOutput").ap()

    with tile.TileContext(nc) as tc:
        with (
            tc.tile_pool(name="weights", bufs=1) as wpool,
            tc.tile_pool(name="xt", bufs=1) as xpool,
            tc.tile_pool(name="acts", bufs=1) as apool,
            tc.tile_pool(name="carry", bufs=2) as cpool,
            tc.tile_pool(name="hts", bufs=3) as hpool,
            tc.tile_pool(name="outs", bufs=3) as opool,
            tc.tile_pool(name="psum", bufs=2, space="PSUM") as ppool,
        ):
            # ---- persistent tiles (3D: [128, d/h-chunk, cols]) ---------
            wih0 = wpool.tile([128, 4, 1024], F8, tag="wih0", name="wih0")
            wih1 = wpool.tile([128, 4, 1536], F8, tag="wih1", name="wih1")
            whh0 = wpool.tile([128, 4, G4], F8, tag="whh0", name="whh0")
            whh1 = wpool.tile([128, 4, G4], F8, tag="whh1", name="whh1")
            wg0 = wpool.tile([128, 4, H], BF, tag="wg0", name="wg0")
            wg1x = wpool.tile([128, 4, H], BF, tag="wg1x", name="wg1x")
            xf8 = xpool.tile([128, 4, TOK], F8, tag="xf8", name="xf8")
            xbf = xpool.tile([128, 4, TOK], BF, tag="xbf", name="xbf")

            # One DMA descriptor per (tensor, k-chunk-pair): the per-chunk
            # scheme needed 60 descriptors at ~607ns serial issue on the
            # sync queue, starving the PE head for ~25us.  A rearranged
            # source AP folds the 128-row chunking into a single 3D
            # descriptor.
            def load_w2(eng, sb, dram, k0):
                eng.dma_start(
                    sb[:, k0:k0 + 2, :],
                    dram[128 * k0:128 * (k0 + 2), :].rearrange(
                        "(k p) c -> p k c", k=2))

            def load_x_blk(eng, sb, dram, b):
                eng.dma_start(
                    sb[:, :, BLK * b:BLK * (b + 1)],
                    dram[:, BLK * b:BLK * (b + 1)].rearrange(
                        "(k p) c -> p k c", k=4))

            # Sync queue, need-by order.  First the block-0 L0 operands at
            # chunk-pair granularity so the kp=0 matmuls' dependencies land
            # first and compute overlaps the rest.
            load_w2(nc.sync, wih0, wih0_d, 0)
            nc.sync.dma_start(
                xf8[:, 0:2, 0:BLK],
                xf8_d[0:256, 0:BLK].rearrange("(k p) c -> p k c", k=2))
            load_w2(nc.sync, wih0, wih0_d, 2)
            nc.sync.dma_start(
                xf8[:, 2:4, 0:BLK],
                xf8_d[256:512, 0:BLK].rearrange("(k p) c -> p k c", k=2))
            for k0 in (0, 2):
                load_w2(nc.sync, wg0, wg0_d, k0)
                nc.sync.dma_start(
                    xbf[:, k0:k0 + 2, 0:BLK],
                    xbf_d[128 * k0:128 * (k0 + 2), 0:BLK].rearrange(
                        "(k p) c -> p k c", k=2))
            load_x_blk(nc.sync, xf8, xf8_d, 1)
            load_x_blk(nc.sync, xbf, xbf_d, 1)
            load_w2(nc.sync, wih1, wih1_d, 0)
            load_w2(nc.sync, wih1, wih1_d, 2)
            load_w2(nc.sync, whh1, whh1_d, 0)
            load_w2(nc.sync, whh1, whh1_d, 2)
            load_w2(nc.sync, wg1x, wg1x_d, 0)
            load_w2(nc.sync, wg1x, wg1x_d, 2)
            load_w2(nc.sync, whh0, whh0_d, 0)
            load_w2(nc.sync, whh0, whh0_d, 2)
            load_x_blk(nc.sync, xf8, xf8_d, 2)
            load_x_blk(nc.sync, xbf, xbf_d, 2)
            load_x_blk(nc.sync, xf8, xf8_d, 3)
            load_x_blk(nc.sync, xbf, xbf_d, 3)

            # ---- PE warm-up -------------------------------------------
            # Trivial bf16 matmuls run while the head DMAs are in flight so
            # the PE clock-gate reaches 8/8 as the first real data lands.
            # Cheap N=64 matmuls (~56-107ns each): the engine-init preamble
            # pins the first DMA issue to ~7.2us and first data to ~9us;
            # a dense warm stream from ~7.5us flips the HAM clock-gate to
            # 8/8 by ~11us so the real stream runs at 2.4 GHz throughout.
            warm = wpool.tile([128, 129], BF, tag="warm", name="warm")
            nc.gpsimd.memset(warm[:], 0.0)
            warm_ps = ppool.tile([128, BLK], mybir.dt.float32, tag="ps", name="ps")
            for _ in range(56):
                nc.tensor.matmul(warm_ps[0:1, 0:64], warm[:, 0:1], warm[:, 1:65],
                                 start=True, stop=True)

            # ---- matmul emitters --------------------------------------
            # fp8 DoubleRow gate matmuls: psum[:, BLK*c:+BLK] accumulates
            # over chunk-pairs kp of (w pair-plane, x pair-plane).
            def dr_gates(psum_t, w, off, rhs, b, do_start=True, do_stop=True):
                for kp in (0, 2):
                    for c in range(4):
                        nc.tensor.matmul(
                            psum_t[:, BLK * c:BLK * (c + 1)],
                            w[:, kp:kp + 2, off + 128 * c: off + 128 * (c + 1)],
                            rhs[:, kp:kp + 2, BLK * b:BLK * (b + 1)],
                            start=(do_start and kp == 0),
                            stop=(do_stop and kp == 2),
                            perf_mode=DR,
                        )

            # bf16 gate matmuls (the tanh gates).
            def bf_gates(psum_t, w, rhs, b, do_start=True, do_stop=True):
                for k in range(4):
                    for c in range(4):
                        nc.tensor.matmul(
                            psum_t[:, BLK * c:BLK * (c + 1)],
                            w[:, k, 128 * c: 128 * (c + 1)],
                            rhs[:, k, BLK * b:BLK * (b + 1)],
                            start=(do_start and k == 0),
                            stop=(do_stop and k == 3),
                        )

            def act_tile(tag, dt=BF):
                return apool.tile([128, 4 * BLK], dt, tag=tag, name=tag)

            # ---- software pipeline ------------------------------------
            # iter it: L0 gates of block it; L1 gates of block it-1 (h0
            # ready); z matmuls + stores of block it-2 (h1 ready).
            #
            # Within an iteration, z-tile emission is INTERLEAVED between
            # gate emissions: a z psum tile fills in ~1.7us but its ACT
            # sigmoid drain takes ~2us (+ queue), so 8 back-to-back z tiles
            # rotating through 2 psum slots stall the PE ~0.3-2us each.
            # Alternating z tiles with >=1.7us gate producers gives every
            # slot >=3.4us to drain.
            h0Fs = [None] * NB   # h0T fp8 [128, 4, BLK]
            h1Fs = [None] * NB
            c0s = [None] * NB

            def new_ps():
                return ppool.tile([128, 4 * BLK], mybir.dt.float32, tag="ps", name="ps")

            def _chain(*fns):
                def run():
                    for f in fns:
                        f()
                return run

            for it in range(NB + 1):
                gate_emits = []
                z_emits = []

                if it < NB:
                    b = it
                    # ---- layer 0 gates (f unused: skipped) ----
                    i0 = act_tile("i0")
                    g0 = act_tile("g0")
                    o0 = act_tile("o0")
                    c0 = cpool.tile([128, 4 * BLK], BF, tag="c0")
                    thc0 = act_tile("thc0")
                    h0F = hpool.tile([128, 4, BLK], F8, tag="h0F")
                    h0Fs[b], c0s[b] = h0F, c0

                    def l0_i(b=b, i0=i0):
                        ps = new_ps()
                        dr_gates(ps, wih0, 0, xf8, b)
                        nc.scalar.activation(i0[:], ps[:], SIG)

                    def l0_g(b=b, g0=g0):
                        ps = new_ps()
                        bf_gates(ps, wg0, xbf, b)
                        nc.scalar.activation(g0[:], ps[:], TANH)

                    def l0_o(b=b, o0=o0):
                        ps = new_ps()
                        dr_gates(ps, wih0, 512, xf8, b)
                        nc.scalar.activation(o0[:], ps[:], SIG)

                    def l0_chain(i0=i0, g0=g0, o0=o0, c0=c0, thc0=thc0,
                                 h0F=h0F):
                        # h0 chain (slack: consumed a full iteration later).
                        nc.vector.tensor_mul(c0[:], i0[:], g0[:])
                        nc.scalar.activation(thc0[:], c0[:], TANH)
                        nc.vector.tensor_mul(
                            h0F[:].rearrange("p a b -> p (a b)"), o0[:], thc0[:])

                    # Block 0: g0's operands (wg0 + xbf) are last to arrive
                    # over the head DMA, so run o0 (no new data) before g0.
                    # The chain emitter must come after BOTH its producers.
                    if it == 0:
                        gate_emits += [l0_i, l0_o, _chain(l0_g, l0_chain)]
                    else:
                        gate_emits += [l0_i, l0_g, _chain(l0_o, l0_chain)]

                if 1 <= it <= NB:
                    b = it - 1
                    h0F, c0 = h0Fs[b], c0s[b]
                    i1 = act_tile("i1")
                    f1 = act_tile("f1")
                    o1 = act_tile("o1")
                    g1 = act_tile("g1")
                    thc1 = act_tile("thc1")
                    c1 = cpool.tile([128, 4 * BLK], BF, tag="c1")
                    h1F = hpool.tile([128, 4, BLK], F8, tag="h1F")
                    h1Fs[b] = h1F

                    # fp8 i/f/o: x-part pairs then h0-part pairs.
                    # h-part: whh1 full layout (i=0,f=512,o=1536)
                    def l1_fp8(at, off, hoff, b=b, h0F=h0F):
                        ps = new_ps()
                        dr_gates(ps, wih1, off, xf8, b, do_stop=False)
                        for kp in (0, 2):
                            for c in range(4):
                                nc.tensor.matmul(
                                    ps[:, BLK * c:BLK * (c + 1)],
                                    whh1[:, kp:kp + 2, hoff + 128 * c: hoff + 128 * (c + 1)],
                                    h0F[:, kp:kp + 2, :],
                                    start=False, stop=(kp == 2),
                                    perf_mode=DR,
                                )
                        nc.scalar.activation(at[:], ps[:], SIG)

                    # NB: everything must be default-bound -- the z section
                    # below re-binds h0F/h1F before these closures run.
                    # g1's h-part runs fp8 DR (whh1 g-columns live at 1024):
                    # sim says this costs +0.8e-3 rel err (1.65e-2 total),
                    # within budget, and saves 8 bf16 matmuls + the h0B mul.
                    def l1_g(b=b, h0F=h0F, g1=g1, i1=i1, f1=f1, c0=c0, c1=c1):
                        ps = new_ps()
                        bf_gates(ps, wg1x, xbf, b, do_stop=False)
                        for kp in (0, 2):
                            for c in range(4):
                                nc.tensor.matmul(
                                    ps[:, BLK * c:BLK * (c + 1)],
                                    whh1[:, kp:kp + 2, 1024 + 128 * c: 1024 + 128 * (c + 1)],
                                    h0F[:, kp:kp + 2, :],
                                    start=False, stop=(kp == 2),
                                    perf_mode=DR,
                                )
                        nc.scalar.activation(g1[:], ps[:], TANH)
                        # c1 = sig(f1)*c0 + sig(i1)*tanh(g1): runs while the
                        # o1 matmuls stream, so h1F is ready ~4us after the
                        # last gate instead of ~12.
                        nc.vector.tensor_mul(f1[:], f1[:], c0[:])
                        nc.vector.tensor_mul(g1[:], i1[:], g1[:])
                        nc.vector.tensor_add(c1[:], f1[:], g1[:])

                    def l1_o(o1=o1, c1=c1, thc1=thc1, h1F=h1F):
                        l1_fp8(o1, 1024, 1536)
                        nc.scalar.activation(thc1[:], c1[:], TANH)
                        nc.vector.tensor_mul(
                            h1F[:].rearrange("p a b -> p (a b)"), o1[:], thc1[:])

                    gate_emits += [
                        lambda i1=i1: l1_fp8(i1, 0, 0),
                        lambda f1=f1: l1_fp8(f1, 512, 512),
                        l1_g,
                        l1_o,
                    ]

                # ---- z matmuls (fp8 DR), [tok, unit] layout + store ----
                # stationary: hT token-chunk pairs; moving: whh unit cols.
                # Split by half: the h0 half of block b runs at it=b+1 (h0F
                # is ready an iteration before h1F), the h1 half at it=b+2.
                # This halves the z-only final iteration and fills the thin
                # early iterations.
                # zh0 first: h0F(b) is produced mid-iteration b, while
                # h1F(b) lands only at the END of iteration b+1 -- so a
                # zh1 tile must never be the first z consumed.
                z_src = []
                if 1 <= it <= NB:
                    z_src.append((it - 1, 0, False))
                if 2 <= it:
                    z_src.append((it - 2, 1, False))
                if it == NB:
                    # Final block's h1 half: h1F(NB-1) completes mid-way
                    # through this very iteration; its tiles run last.
                    z_src.append((it - 1, 1, True))
                z_late = []
                for j in range(4):  # 128-token chunks within block
                    for b, half, late in z_src:
                        hT = h0Fs[b] if half == 0 else h1Fs[b]
                        w = whh0 if half == 0 else whh1
                        rows = out_d[BLK * b + 128 * j: BLK * b + 128 * (j + 1), :]
                        if True:
                            last = (late and j == 3)
                            if not last:
                                # (gpsimd SWDGE stores tried here: its
                                # queue drain costs ~5us before the final
                                # barrier -- sync is net faster.)
                                seng = nc.sync

                                def z_tile(rows=rows, half=half, hT=hT, w=w,
                                           j=j, seng=seng):
                                    ps = new_ps()
                                    for kp in (0, 2):
                                        lhsT = hT[:, kp:kp + 2, 128 * j: 128 * (j + 1)]
                                        for n in range(4):
                                            nc.tensor.matmul(
                                                ps[:, 512 * n:512 * (n + 1)],
                                                lhsT,
                                                w[:, kp:kp + 2, 512 * n: 512 * (n + 1)],
                                                start=(kp == 0), stop=(kp == 2),
                                                perf_mode=DR,
                                            )
                                    ot = opool.tile([128, G4], mybir.dt.float32, tag="ot", name="ot")
                                    nc.scalar.activation(ot[:], ps[:], SIG)
                                    seng.dma_start(
                                        rows[:, G4 * half:G4 * (half + 1)], ot[:])
                            else:
                                # very last tile: 4 separate 512-wide psum
                                # tiles with per-slice sigmoid+store so the
                                # kernel tail drains after a 512-wide slice.
                                def z_tile(rows=rows, half=half, hT=hT, w=w, j=j):
                                    ot = opool.tile([128, G4], mybir.dt.float32, tag="ot", name="ot")
                                    for n in range(4):
                                        psn = ppool.tile([128, BLK], mybir.dt.float32, tag="ps", name="ps")
                                        for kp in (0, 2):
                                            lhsT = hT[:, kp:kp + 2, 128 * j: 128 * (j + 1)]
                                            nc.tensor.matmul(
                                                psn[:],
                                                lhsT,
                                                w[:, kp:kp + 2, 512 * n: 512 * (n + 1)],
                                                start=(kp == 0), stop=(kp == 2),
                                                perf_mode=DR,
                                            )
                                        sl = slice(512 * n, 512 * (n + 1))
                                        nc.scalar.activation(ot[:, sl], psn[:], SIG)
                                        nc.sync.dma_start(
                                            rows[:, G4 * half + 512 * n: G4 * half + 512 * (n + 1)],
                                            ot[:, sl])
                            (z_late if late else z_emits).append(z_tile)

                # Emission order: L0 gates FIRST so the h0 chain (i0/g0
                # ACTs -> c0 -> thc0 -> h0F) starts as early as possible in
                # the saturated ACT FIFO -- the next iteration's L1 h-part
                # matmuls depend on it.  Then L1 gates, each followed by two
                # z tiles: alternating >=3.4us gate producers with 1.7us z
                # producers gives every psum slot pair enough drain time.
                # The final block's h1-half tiles (z_late) go at the very
                # end, covered by the regular z leftovers while the c1/h1F
                # chain completes.
                n_l0 = 3 if it < NB else 0
                if it == NB:
                    # Final iteration: one zh0 lead tile; keep the gate ACTs
                    # early in the ACT FIFO (few z ACTs before thc1) so the
                    # h1F chain finishes before the trailing late tiles; the
                    # z backlog runs after o1 while the chain completes.
                    g = gate_emits
                    order = ([z_emits[0], g[0], g[1], z_emits[1], g[2],
                              z_emits[2], g[3]] + z_emits[3:] + z_late)
                else:
                    order = list(gate_emits[:n_l0])
                    zi = 0
                    for ge in gate_emits[n_l0:]:
                        order.append(ge)
                        order += z_emits[zi:zi + 2]
                        zi += 2
                    order += z_emits[zi:]
                for e in order:
                    e()

    nc.compile()
    return nc


_NC = None


def _get_nc():
    global _NC
    if _NC is None:
        _NC = _build()
    return _NC


def kernel(input_noise, W_ih, W_hh, b_ih, b_hh):
    input_noise = np.asarray(input_noise)
    W_ih = np.asarray(W_ih)
    W_hh = np.asarray(W_hh)

    # Host-side prep: transpose + cast (negligible vs device work).
    # Gate column order is i, f, g, o (H columns each) in the 4H dim.
    wih0T = np.ascontiguousarray(W_ih[0].T)                # [D, 4H] fp32
    wih1T = np.ascontiguousarray(W_ih[1].T)
    whh0T = np.ascontiguousarray(W_hh[0].T)                # [H, 4H]
    whh1T = np.ascontiguousarray(W_hh[1].T)

    wih0 = np.ascontiguousarray(
        np.concatenate([wih0T[:, 0:H], wih0T[:, 3 * H:]], axis=1)).astype(F8_NP)
    wih1 = np.ascontiguousarray(
        np.concatenate([wih1T[:, 0:2 * H], wih1T[:, 3 * H:]], axis=1)).astype(F8_NP)
    whh0 = whh0T.astype(F8_NP)
    whh1 = whh1T.astype(F8_NP)
    wg0 = np.ascontiguousarray(wih0T[:, 2 * H:3 * H]).astype(BF_NP)
    wg1x = np.ascontiguousarray(wih1T[:, 2 * H:3 * H]).astype(BF_NP)

    xs = input_noise.reshape(NCORES, TOK, D)               # batch-sharded
    in_maps = []
    for c in range(NCORES):
        xT = np.ascontiguousarray(xs[c].T)                 # [D, TOK]
        in_maps.append({"xf8": xT.astype(F8_NP), "xbf": xT.astype(BF_NP),
                        "wih0": wih0, "wih1": wih1,
                        "whh0": whh0, "whh1": whh1,
                        "wg0": wg0, "wg1x": wg1x})

    nc = _get_nc()
    trace = bool(int(os.environ.get("TRNK_TRACE", "0")))
    if trace:
        try:
            import trnprof  # noqa: F401  (installs the axon NTFF hook)
        except ImportError:
            trace = False
    res = run_bass_kernel_spmd(nc, in_maps, core_ids=list(range(NCORES)),
                               trace=trace)
    if trace:
        kernel.last_exec_time_ns = res.exec_time_ns
        kernel.last_trace = (res.instructions_and_trace or (None, None))[1]
    out = np.stack([res.results[c]["out"] for c in range(NCORES)])
    return out.reshape(B, T, 2 * G4)
